# revision 9
# baseline (speedup 1.0000x reference)
"""Trainium2 Bass kernel v2 for nn_GTShapelet (GIN stack + CLS-query MHA).

Structure change vs v1: instead of AllGather-ing full h between GIN layers
(2 x 265us collectives), messages are computed source-sharded and combined
with ReduceScatter (output 1/8 the size -> ~4x cheaper collective):

  - Each core owns nodes [c*4096, (c+1)*4096) and the edges whose SOURCE
    lies in its range.
  - Layer algebra: (h + segsum(ew*h[src])) @ W + b
      = g_own + segsum(ew*g[src]) + b', where g := h @ W (+bias folded).
    So each layer's exchange carries the PRE-PROJECTED g rows; after the
    ReduceScatter only an elementwise add + gelu remains.
  - Per layer: gather own-edge source rows from the core's own g table in
    HBM (512B/desc), selection-matmul them into 128-dst-chunk psums
    (contiguous slot packing, boundary-spanning batches), write bf16
    partials [32768, F] stripe-major, ReduceScatter(add) in 4 stripes
    overlapped with compute.
  - L1 stays the dense vocab-matmul (ct @ T1) since node ids live in a
    1024-row table; W2 is fused right after per stripe.
"""

import sys

if "/opt/trn_rl_repo" not in sys.path:
    sys.path.insert(0, "/opt/trn_rl_repo")

import numpy as np
import ml_dtypes  # noqa: F401

# ---- problem constants (hardcoded per spec) ----
B, N, E, D = 32, 1024, 524288, 128
H, HD = 4, 32
F2 = 2 * D                      # 256
NCORES = 8
NPC = B * N // NCORES           # 4096 own nodes per core
GPC = B // NCORES               # 4 graphs per core
CH = 128                        # dst chunk width (smaj positions)
NCH = B * N // CH               # 256 chunks globally
WCH = 8                         # chunks per gather window
NW = NCH // WCH                 # 16 windows
NSTR = 4                        # ReduceScatter stripes
OWN_STR = NPC // NSTR           # 1024 own rows per stripe
STR_ROWS = B * N // NSTR        # 8192 smaj rows per stripe
DH, DL = 16, 8                  # chunk one-hot factorization: d = 8*dh + dl
DSENT = 200.0
BF16 = np.dtype('bfloat16')

_prog_cache = {}


def _smaj_of(d):
    return ((d % NPC) // OWN_STR * (NCORES * OWN_STR)
            + (d // NPC) * OWN_STR + (d % OWN_STR))


def _build_plan(src, dst):
    """Static slot/pair plan shared by all cores (max-over-cores caps)."""
    src = np.asarray(src).astype(np.int64)
    dst = np.asarray(dst).astype(np.int64)
    smaj = _smaj_of(dst)
    core_of = src // NPC
    counts = np.zeros((NCORES, NCH), np.int64)
    eidx = []
    for c in range(NCORES):
        m = np.nonzero(core_of == c)[0]
        e = m[np.argsort(smaj[m], kind='stable')]
        eidx.append(e)
        counts[c] = np.bincount(smaj[e] // CH, minlength=NCH)
    caps = counts.max(axis=0)
    slot_off = np.zeros(NCH + 1, np.int64)
    wcap = np.zeros(NW, np.int64)
    wbase = np.zeros(NW, np.int64)
    off = 0
    for w in range(NW):
        wbase[w] = off
        for k in range(w * WCH, (w + 1) * WCH):
            slot_off[k] = off
            off += caps[k]
        off = (off + 127) // 128 * 128
        wcap[w] = off - wbase[w]
    slot_off[NCH] = off
    SL = off
    # pairs: (w, batch, chunk); per-window local enumeration
    wpairs = []          # per window: list of (b_local, k)
    for w in range(NW):
        pw = []
        for b in range(wbase[w] // 128, (wbase[w] + wcap[w]) // 128):
            blo, bhi = b * 128, b * 128 + 128
            for k in range(w * WCH, (w + 1) * WCH):
                klo, khi = slot_off[k], slot_off[k] + caps[k]
                if klo < bhi and blo < khi:
                    pw.append((b - wbase[w] // 128, k))
        wpairs.append(pw)
    PW = max(len(pw) for pw in wpairs)
    return dict(caps=caps, slot_off=slot_off, wcap=wcap, wbase=wbase,
                SL=SL, wpairs=wpairs, PW=PW, eidx=eidx, smaj=smaj)


def _build_program(plan, variant="hw"):
    key = (variant, plan['SL'], plan['PW'])
    if key in _prog_cache:
        return _prog_cache[key]
    import concourse.bacc as bacc
    import concourse.tile as tile
    import concourse.mybir as mybir
    from concourse.library_config import mlp

    dt = mybir.dt
    AF = mybir.ActivationFunctionType
    OP = mybir.AluOpType
    AF_GELU = AF.Identity if variant == "sim" else AF.Gelu

    SL = int(plan['SL'])
    PW = int(plan['PW'])
    NPAD = NW * PW
    wcap = [int(x) for x in plan['wcap']]
    wbase = [int(x) for x in plan['wbase']]
    wpairs = plan['wpairs']
    NBMAX = max(wcap) // 128

    nc = bacc.Bacc("TRN2", target_bir_lowering=False, debug=False,
                   num_devices=NCORES)

    def din(name, shape, dtype):
        return nc.dram_tensor(name, shape, dtype, kind="ExternalInput")

    t1 = din("t1", [N, F2], dt.bfloat16)
    h0w1own = din("h0w1own", [NPC, F2], dt.bfloat16)
    ct = din("ct", [128, N // 128, NPC], dt.bfloat16)
    b1 = din("b1", [128, 2], dt.float32)
    w2t = din("w2t", [128, 4 * 128], dt.bfloat16)
    b2 = din("b2", [128, 2], dt.float32)
    w3t = din("w3t", [128, 2 * 128], dt.bfloat16)
    b3 = din("b3", [128, 1], dt.float32)
    idx12 = din("idx12", [128, SL // 16], dt.int16)
    dsth = din("dsth", [128, NPAD], dt.bfloat16)
    dstl8 = din("dstl8", [128, NPAD], dt.bfloat16)
    eww = din("eww", [128, NPAD], dt.bfloat16)
    iotah = din("iotah", [128, DH * PW], dt.bfloat16)
    iotal = din("iotal", [128, DL * PW], dt.bfloat16)
    i64 = din("i64", [128, 64], dt.bfloat16)
    i128 = din("i128", [128, 128], dt.bfloat16)
    i128f = din("i128f", [128, 128], dt.float32)
    i4 = din("i4", [4, 4], dt.bfloat16)
    wk = din("wk", [128, 128], dt.bfloat16)
    bk = din("bk", [128, 1], dt.float32)
    wv = din("wv", [128, 128], dt.bfloat16)
    bv = din("bv", [128, 1], dt.float32)
    qblk = din("qblk", [128, 4], dt.bfloat16)
    vc4 = din("vc4", [4, 128], dt.bfloat16)
    e4 = din("e4", [4, 4], dt.bfloat16)
    msel = din("msel", [128, 4], dt.float32)
    r4 = din("r4", [4, 128], dt.float32)
    ecls = din("ecls", [4, 1], dt.float32)
    eps = din("eps", [4, 1], dt.float32)
    ynb = din("ynb", [128, 1], dt.float32)
    wo = din("wo", [128, 128], dt.bfloat16)
    lng = din("lng", [4, 128], dt.float32)
    lnb = din("lnb", [4, 128], dt.float32)
    y_out = nc.dram_tensor("y", [GPC, D], dt.float32, kind="ExternalOutput")

    with tile.TileContext(nc) as tc:
        nc.gpsimd.load_library(mlp)
        with tc.tile_pool(name="const", bufs=1) as cp, \
             tc.tile_pool(name="res", bufs=1) as rp, \
             tc.tile_pool(name="work", bufs=1) as wp, \
             tc.tile_pool(name="dram", bufs=1, space="DRAM") as dram:

            def cload(ap, shape, dtype):
                t = cp.tile(shape, dtype, name=f"c_{ap.name}")
                nc.sync.dma_start(out=t[:], in_=ap[:])
                return t

            b1_t = cload(b1, [128, 2], dt.float32)
            w2t_t = cload(w2t, [128, 4 * 128], dt.bfloat16)
            b2_t = cload(b2, [128, 2], dt.float32)
            w3t_t = cload(w3t, [128, 2 * 128], dt.bfloat16)
            b3_t = cload(b3, [128, 1], dt.float32)
            i64_t = cload(i64, [128, 64], dt.bfloat16)
            i128_t = cload(i128, [128, 128], dt.bfloat16)
            i128f_t = cload(i128f, [128, 128], dt.float32)
            i4_t = cload(i4, [4, 4], dt.bfloat16)
            wk_t = cload(wk, [128, 128], dt.bfloat16)
            bk_t = cload(bk, [128, 1], dt.float32)
            wv_t = cload(wv, [128, 128], dt.bfloat16)
            bv_t = cload(bv, [128, 1], dt.float32)
            qblk_t = cload(qblk, [128, 4], dt.bfloat16)
            vc4_t = cload(vc4, [4, 128], dt.bfloat16)
            e4_t = cload(e4, [4, 4], dt.bfloat16)
            msel_t = cload(msel, [128, 4], dt.float32)
            r4_t = cload(r4, [4, 128], dt.float32)
            ecls_t = cload(ecls, [4, 1], dt.float32)
            eps_t = cload(eps, [4, 1], dt.float32)
            ynb_t = cload(ynb, [128, 1], dt.float32)
            wo_t = cload(wo, [128, 128], dt.bfloat16)
            lng_t = cload(lng, [4, 128], dt.float32)
            lnb_t = cload(lnb, [4, 128], dt.float32)

            # persistent SBUF (feature-major g tables for post-RS adds)
            g1T = rp.tile([128, 2, NPC], dt.bfloat16, name="g1T")
            g2T = rp.tile([128, NPC], dt.bfloat16, name="g2T")
            h3T = rp.tile([128, NPC], dt.bfloat16, name="h3T")

            # DRAM scratch
            g1own = dram.tile([NPC, F2], dt.bfloat16, tag="g1own")
            g2own = dram.tile([NPC, D], dt.bfloat16, tag="g2own")
            part2 = dram.tile([B * N, F2], dt.bfloat16, tag="part2")
            msg2 = dram.tile([NPC, F2], dt.bfloat16, tag="msg2")
            part3 = dram.tile([B * N, D], dt.bfloat16, tag="part3")
            msg3 = dram.tile([NPC, D], dt.bfloat16, tag="msg3")

            g1own_v = g1own.rearrange("(t p) f -> p t f", p=128)
            g2own_v = g2own.rearrange("(t p) f -> p t f", p=128)

            # ================= L1 + fused W2 =================
            with tc.tile_pool(name="l1_ps", bufs=1, space="PSUM") as pp, \
                 tc.tile_pool(name="l1_sb", bufs=1) as gp:
                t1sb = gp.tile([128, N // 128, F2], dt.bfloat16, name="t1sb")
                nc.sync.dma_start(
                    out=t1sb[:],
                    in_=t1.rearrange("(kk p) f -> p kk f", p=128))
                hown0 = gp.tile([128, NPC // 128, F2], dt.bfloat16,
                                name="hown0")
                nc.sync.dma_start(
                    out=hown0[:],
                    in_=h0w1own.rearrange("(t p) f -> p t f", p=128))
                hT1 = gp.tile([128, 2, NPC], dt.bfloat16, name="hT1")
                ctbufs = [gp.tile([128, N // 128, 512], dt.bfloat16,
                                  name=f"ctb{i}") for i in range(2)]
                def w2_phase(s):
                    # W2 for the 512 nodes of ct-segment s (chunks 8s..8s+8)
                    nsl = slice(s * 512, (s + 1) * 512)
                    for jo in range(2):
                        ps2 = pp.tile([128, 512], dt.float32, tag="w2p",
                                      bufs=2)
                        for ji in range(2):
                            nc.tensor.matmul(
                                out=ps2[:],
                                lhsT=w2t_t[:, (2 * ji + jo) * 128:
                                           (2 * ji + jo + 1) * 128],
                                rhs=hT1[:, ji, nsl],
                                start=(ji == 0), stop=(ji == 1))
                        nc.scalar.activation(
                            g1T[:, jo, nsl], ps2[:],
                            AF.Identity, bias=b2_t[:, jo:jo + 1])
                    g1st = gp.tile([128, 4, F2], dt.bfloat16, tag="g1st",
                                   bufs=2)
                    for t in range(4):
                        for j in range(2):
                            tp2 = pp.tile([128, 128], dt.bfloat16, tag="tp2",
                                          bufs=2)
                            nc.tensor.transpose(
                                tp2[:],
                                g1T[:, j, s * 512 + t * 128:
                                    s * 512 + (t + 1) * 128],
                                i128_t[:])
                            if (t + j) % 2 == 0:
                                nc.vector.tensor_copy(
                                    out=g1st[:, t, j * 128:(j + 1) * 128],
                                    in_=tp2[:])
                            else:
                                nc.scalar.activation(
                                    g1st[:, t, j * 128:(j + 1) * 128],
                                    tp2[:], AF.Copy)
                    nc.sync.dma_start(
                        out=g1own_v[:, s * 4:(s + 1) * 4, :], in_=g1st[:])

                for s in range(8):
                    ctb = ctbufs[s % 2]
                    nc.sync.dma_start(
                        out=ctb[:], in_=ct[:, :, s * 512:(s + 1) * 512])
                    for kk in range(8):
                        k = s * 8 + kk
                        ps = pp.tile([64, F2], dt.float32, tag="l1p",
                                     bufs=2)
                        for kt in range(N // 128):
                            nc.tensor.matmul(
                                out=ps[:],
                                lhsT=ctb[:, kt, kk * 64:(kk + 1) * 64],
                                rhs=t1sb[:, kt, :],
                                start=(kt == 0), stop=False)
                        half = (k % 2) * 64
                        nc.tensor.matmul(
                            out=ps[:], lhsT=i64_t[half:half + 64, :],
                            rhs=hown0[half:half + 64, k // 2, :],
                            start=False, stop=True)
                        # b1 is folded into h0w1own on the host, so the
                        # psum drain IS the gelu; transposed copies are
                        # then plain copies split across DVE/ACT
                        msb = gp.tile([64, F2], dt.bfloat16, tag="msb",
                                      bufs=3)
                        nc.scalar.activation(msb[:], ps[:], AF_GELU)
                        for j in range(2):
                            tp = pp.tile([128, 64], dt.bfloat16, tag="tp",
                                         bufs=2)
                            nc.tensor.transpose(
                                tp[:], msb[:, j * 128:(j + 1) * 128],
                                i64_t[0:64, :])
                            if (2 * k + j) % 2 == 0:
                                nc.vector.tensor_copy(
                                    out=hT1[:, j, k * 64:(k + 1) * 64],
                                    in_=tp[:])
                            else:
                                nc.scalar.activation(
                                    hT1[:, j, k * 64:(k + 1) * 64],
                                    tp[:], AF.Copy)
                    # issue previous segment's W2 phase behind this one's
                    # chunks so its latency chain hides under ct matmuls
                    if s >= 1:
                        w2_phase(s - 1)
                w2_phase(7)

            # exchange consts load late so they overlap L1 compute
            idx12_t = cload(idx12, [128, SL // 16], dt.int16)
            dsth_t = cload(dsth, [128, NPAD], dt.bfloat16)
            dstl8_t = cload(dstl8, [128, NPAD], dt.bfloat16)
            eww_t = cload(eww, [128, NPAD], dt.bfloat16)
            iotah_t = cload(iotah, [128, DH * PW], dt.bfloat16)
            iotal_t = cload(iotal, [128, DL * PW], dt.bfloat16)

            # attention state shared across L3 stripes
            kT = rp.tile([128, NPC], dt.bfloat16, name="kT")
            vnm = rp.tile([128, NPC // 128, 128], dt.bfloat16, name="vnm")
            ctx_all = rp.tile([128, 4], dt.bfloat16, name="ctx_all")

            # ============ exchange layers (L2: F2-wide, L3: D-wide) ============
            def exchange_layer(F, gsrc, part, msg, post_stripe, pwait):
                with tc.tile_pool(name="ex_ps", bufs=1, space="PSUM") as pp, \
                     tc.tile_pool(name="ex_sb", bufs=1) as xp:
                    gbufs = [xp.tile([128, NBMAX, F], dt.bfloat16,
                                     name=f"gb{i}") for i in range(3)]
                    svbufs = [xp.tile([128, CH * PW], dt.bfloat16,
                                      name=f"sv{i}") for i in range(3)]
                    stbufs = [xp.tile([128, WCH, F], dt.bfloat16,
                                      name=f"st{i}") for i in range(2)]
                    part_v = part.rearrange("(w q p) f -> p w q f", p=128,
                                            w=NW)

                    def issue_rs(sidx):
                        nc.gpsimd.collective_compute(
                            "ReduceScatter", OP.add,
                            replica_groups=[list(range(NCORES))],
                            ins=[part[sidx * STR_ROWS:
                                      (sidx + 1) * STR_ROWS, :].opt()],
                            outs=[msg[sidx * OWN_STR:
                                      (sidx + 1) * OWN_STR, :].opt()])

                    for w in range(NW):
                        nb = wcap[w] // 128
                        gb = gbufs[w % 3]
                        # sub-calls of <=1024 descriptors: the SWDGE ring
                        # holds 1024 entries, larger single calls overflow it
                        for b0 in range(0, nb, 8):
                            nsub = min(8, nb - b0) * 128
                            base = wbase[w] + b0 * 128
                            nc.gpsimd.dma_gather(
                                gb[:, b0:b0 + nsub // 128, :], gsrc[:],
                                idx12_t[:, base // 16:(base + nsub) // 16],
                                nsub, nsub, F)
                        # S = onehot(dsth) x (onehot(dstl8) * ew), d=8*dh+dl
                        svh = xp.tile([128, DH, PW], dt.bfloat16, tag="svh",
                                      bufs=2)
                        svl = xp.tile([128, DL, PW], dt.bfloat16, tag="svl",
                                      bufs=2)
                        nc.vector.tensor_tensor(
                            out=svh[:],
                            in0=dsth_t[:, w * PW:(w + 1) * PW].unsqueeze(1)
                                .broadcast_to([128, DH, PW]),
                            in1=iotah_t[:].rearrange("p (dh j) -> p dh j",
                                                     j=PW),
                            op=OP.is_equal)
                        nc.vector.tensor_tensor(
                            out=svl[:],
                            in0=dstl8_t[:, w * PW:(w + 1) * PW].unsqueeze(1)
                                .broadcast_to([128, DL, PW]),
                            in1=iotal_t[:].rearrange("p (dl j) -> p dl j",
                                                     j=PW),
                            op=OP.is_equal)
                        nc.vector.tensor_tensor(
                            out=svl[:], in0=svl[:],
                            in1=eww_t[:, w * PW:(w + 1) * PW].unsqueeze(1)
                                .broadcast_to([128, DL, PW]),
                            op=OP.mult)
                        sv = svbufs[w % 3]
                        nc.vector.tensor_tensor(
                            out=sv[:].rearrange("p (dh dl j) -> p dh dl j",
                                                dl=DL, j=PW),
                            in0=svh[:].unsqueeze(2)
                                .broadcast_to([128, DH, DL, PW]),
                            in1=svl[:].unsqueeze(1)
                                .broadcast_to([128, DH, DL, PW]),
                            op=OP.mult)
                        svv = sv[:].rearrange("p (d j) -> p d j", j=PW)
                        st = stbufs[w % 2]
                        for kk2 in range(WCH // 2):
                            ps = pp.tile([128, 2 * F], dt.float32, tag="selp",
                                         bufs=2)
                            for half in range(2):
                                kk = kk2 * 2 + half
                                k = w * WCH + kk
                                prs = [(jl, bl) for jl, (bl, kq) in
                                       enumerate(wpairs[w]) if kq == k]
                                for i, (jl, bl) in enumerate(prs):
                                    nc.tensor.matmul(
                                        out=ps[:, half * F:(half + 1) * F],
                                        lhsT=svv[:, :, jl],
                                        rhs=gb[:, bl, :],
                                        start=(i == 0),
                                        stop=(i == len(prs) - 1))
                            psv3 = ps[:].rearrange("p (q f) -> p q f", q=2)
                            if kk2 % 2 == 0:
                                nc.scalar.activation(
                                    st[:, kk2 * 2:kk2 * 2 + 2, :], psv3,
                                    AF.Copy)
                            else:
                                nc.vector.tensor_copy(
                                    out=st[:, kk2 * 2:kk2 * 2 + 2, :],
                                    in_=psv3)
                        nc.sync.dma_start(out=part_v[:, w, :, :], in_=st[:])
                        # deferred issue: RS for stripe s-1 goes out 2 windows
                        # into stripe s (its pwrite wait is then ~done); the
                        # post-work for stripe s-2 goes out mid stripe s (its
                        # RS finished during stripe s-1) - so no instruction
                        # ever stalls an in-order engine queue that feeds the
                        # gather/matmul pipeline
                        wps = NW // NSTR
                        if w % wps == 2 and w > wps:
                            issue_rs(w // wps - 1)
                        if w % wps == 4 and w > 2 * wps:
                            sp = w // wps - 2
                            with tc.tile_wait_until(
                                    pwait[0] + pwait[1] * sp,
                                    enable=pwait[0] > 0):
                                post_stripe(sp, pp, xp)
                    issue_rs(NSTR - 1)
                    for sp in (NSTR - 2, NSTR - 1):
                        with tc.tile_wait_until(pwait[0] + pwait[1] * sp,
                                                enable=pwait[0] > 0):
                            post_stripe(sp, pp, xp)

            def post_stripe_l2(s, pp, xp):
                # msg readback lands feature-major via XBAR transpose-DMA
                u2T = xp.tile([128, 2, OWN_STR], dt.bfloat16, tag="u2T",
                              bufs=1)
                for j in range(2):
                    nc.sync.dma_start_transpose(
                        u2T[:, j, :],
                        msg2[s * OWN_STR:(s + 1) * OWN_STR,
                             j * 128:(j + 1) * 128])
                nsl = slice(s * OWN_STR, (s + 1) * OWN_STR)
                z2T = xp.tile([128, 2, OWN_STR], dt.bfloat16, tag="z2T",
                              bufs=1)
                nc.vector.tensor_tensor(out=z2T[:], in0=u2T[:],
                                        in1=g1T[:, :, nsl], op=OP.add)
                h2T_st = xp.tile([128, 2, OWN_STR], dt.bfloat16, tag="h2T",
                                 bufs=1)
                nc.scalar.activation(h2T_st[:], z2T[:], AF_GELU)
                for m in range(2):
                    ps3 = pp.tile([128, 512], dt.float32, tag="w3p", bufs=2)
                    for ji in range(2):
                        nc.tensor.matmul(
                            out=ps3[:],
                            lhsT=w3t_t[:, ji * 128:(ji + 1) * 128],
                            rhs=h2T_st[:, ji, m * 512:(m + 1) * 512],
                            start=(ji == 0), stop=(ji == 1))
                    nc.scalar.activation(
                        g2T[:, s * OWN_STR + m * 512:
                            s * OWN_STR + (m + 1) * 512],
                        ps3[:], AF.Identity, bias=b3_t[:, 0:1])
                g2st = xp.tile([128, 8, D], dt.bfloat16, tag="g2st", bufs=2)
                for t in range(8):
                    tp = pp.tile([128, 128], dt.bfloat16, tag="xtp", bufs=2)
                    nc.tensor.transpose(
                        tp[:],
                        g2T[:, s * OWN_STR + t * 128:
                            s * OWN_STR + (t + 1) * 128],
                        i128_t[:])
                    if t % 2 == 0:
                        nc.vector.tensor_copy(out=g2st[:, t, :], in_=tp[:])
                    else:
                        nc.scalar.activation(g2st[:, t, :], tp[:], AF.Copy)
                nc.sync.dma_start(out=g2own_v[:, s * 8:(s + 1) * 8, :],
                                  in_=g2st[:])

            def post_stripe_l3(s, pp, xp):
                u3T = xp.tile([128, OWN_STR], dt.bfloat16, tag="u3T", bufs=1)
                nc.sync.dma_start_transpose(
                    u3T[:], msg3[s * OWN_STR:(s + 1) * OWN_STR, :])
                nsl = slice(s * OWN_STR, (s + 1) * OWN_STR)
                z3T = xp.tile([128, OWN_STR], dt.bfloat16, tag="z3T", bufs=1)
                nc.vector.tensor_tensor(out=z3T[:], in0=u3T[:],
                                        in1=g2T[:, nsl], op=OP.add)
                nc.scalar.activation(h3T[:, nsl], z3T[:], AF_GELU)
                # attention for graph g == stripe s (graphs are 1024 nodes)
                g = s
                for m2 in range(2):
                    nsl = slice(s * 1024 + m2 * 512, s * 1024 + (m2 + 1) * 512)
                    psk = pp.tile([128, 512], dt.float32, tag="psk", bufs=1)
                    nc.tensor.matmul(out=psk[:], lhsT=wk_t[:], rhs=h3T[:, nsl])
                    nc.vector.tensor_scalar(
                        out=kT[:, nsl], in0=psk[:],
                        scalar1=bk_t[:], scalar2=None, op0=OP.add)
                for t in range(8):
                    psv = pp.tile([128, 128], dt.float32, tag="psv", bufs=1)
                    nc.tensor.matmul(
                        out=psv[:],
                        lhsT=h3T[:, (s * 8 + t) * 128:(s * 8 + t + 1) * 128],
                        rhs=wv_t[:])
                    nc.scalar.activation(vnm[:, s * 8 + t, :], psv[:], AF.Copy)
                expt = xp.tile([4, 1024], dt.bfloat16, tag="expt", bufs=1)
                sums2 = xp.tile([4, 2], dt.float32, tag="sums2", bufs=1)
                for hh in range(2):
                    ssc = pp.tile([4, 512], dt.float32, tag="ssc", bufs=1)
                    nc.tensor.matmul(
                        out=ssc[:], lhsT=qblk_t[:],
                        rhs=kT[:, g * 1024 + hh * 512:
                               g * 1024 + (hh + 1) * 512])
                    nc.scalar.activation(expt[:, hh * 512:(hh + 1) * 512],
                                         ssc[:], AF.Exp,
                                         accum_out=sums2[:, hh:hh + 1])
                sums = xp.tile([4, 1], dt.float32, tag="sums", bufs=1)
                nc.vector.tensor_add(out=sums[:], in0=sums2[:, 0:1],
                                     in1=sums2[:, 1:2])
                nc.vector.tensor_add(out=sums[:], in0=sums[:], in1=ecls_t[:])
                psr = pp.tile([128, 1], dt.float32, tag="ptiny", bufs=1)
                nc.tensor.matmul(out=psr[:], lhsT=r4_t[:], rhs=sums[:])
                rbc = xp.tile([128, 1], dt.float32, tag="rbc", bufs=1)
                nc.vector.reciprocal(rbc[:], psr[:])
                psctx = pp.tile([128, 4], dt.float32, tag="psctx", bufs=1)
                for t in range(8):
                    pst = pp.tile([128, 4], dt.bfloat16, tag="ptiny", bufs=1)
                    nc.tensor.transpose(
                        pst[:], expt[:, t * 128:(t + 1) * 128], i4_t[:])
                    ets = xp.tile([128, 4], dt.bfloat16, tag="ets", bufs=1)
                    nc.vector.tensor_copy(out=ets[:], in_=pst[:])
                    nc.tensor.matmul(out=psctx[:],
                                     lhsT=vnm[:, g * 8 + t, :], rhs=ets[:],
                                     start=(t == 0), stop=False)
                nc.tensor.matmul(out=psctx[:], lhsT=vc4_t[:], rhs=e4_t[:],
                                 start=False, stop=True)
                tmp4 = xp.tile([128, 4], dt.float32, tag="tmp4", bufs=1)
                nc.vector.tensor_tensor(out=tmp4[:], in0=psctx[:],
                                        in1=msel_t[:], op=OP.mult)
                ctxv = xp.tile([128, 1], dt.float32, tag="ctxv", bufs=1)
                nc.vector.reduce_sum(out=ctxv[:], in_=tmp4[:],
                                     axis=mybir.AxisListType.X)
                nc.vector.tensor_scalar(out=ctxv[:], in0=ctxv[:],
                                        scalar1=rbc[:], scalar2=bv_t[:],
                                        op0=OP.mult, op1=OP.add)
                nc.vector.tensor_copy(out=ctx_all[:, g:g + 1], in_=ctxv[:])

            exchange_layer(F2, g1own, part2, msg2, post_stripe_l2,
                           (0.227, 0.041))
            exchange_layer(D, g2own, part3, msg3, post_stripe_l3,
                           (0.48, 0.036))

            # ============ final out-projection + layernorm ============
            with tc.tile_pool(name="att_ps", bufs=1, space="PSUM") as ap_, \
                 tc.tile_pool(name="att_sb", bufs=1) as asb:
                psao = ap_.tile([128, 4], dt.float32, tag="ptiny", bufs=2)
                nc.tensor.matmul(out=psao[:], lhsT=wo_t[:], rhs=ctx_all[:])
                ysb = asb.tile([128, 4], dt.float32, tag="ysb")
                nc.vector.tensor_scalar(out=ysb[:], in0=psao[:],
                                        scalar1=ynb_t[:], scalar2=None,
                                        op0=OP.add)
                psy = ap_.tile([4, 128], dt.float32, tag="ptiny", bufs=2)
                nc.tensor.matmul(out=psy[:], lhsT=ysb[:], rhs=i128f_t[:],
                                 is_transpose=True)
                yt = asb.tile([4, 128], dt.float32, tag="yt")
                nc.vector.tensor_copy(out=yt[:], in_=psy[:])
                mn = asb.tile([4, 1], dt.float32, tag="mn")
                nc.vector.reduce_sum(out=mn[:], in_=yt[:],
                                     axis=mybir.AxisListType.X)
                nc.vector.tensor_scalar(out=mn[:], in0=mn[:],
                                        scalar1=1.0 / D, scalar2=None,
                                        op0=OP.mult)
                xc = asb.tile([4, 128], dt.float32, tag="xc")
                nc.vector.tensor_scalar(out=xc[:], in0=yt[:], scalar1=mn[:],
                                        scalar2=None, op0=OP.subtract)
                sq = asb.tile([4, 128], dt.float32, tag="sq")
                ss = asb.tile([4, 1], dt.float32, tag="ss")
                nc.scalar.activation(sq[:], xc[:], AF.Square, accum_out=ss[:])
                sd = asb.tile([4, 1], dt.float32, tag="sd")
                nc.scalar.activation(sd[:], ss[:], AF.Sqrt, bias=eps_t[:],
                                     scale=1.0 / D)
                rr = asb.tile([4, 1], dt.float32, tag="rr")
                nc.vector.reciprocal(rr[:], sd[:])
                yn = asb.tile([4, 128], dt.float32, tag="yn")
                nc.vector.tensor_scalar(out=yn[:], in0=xc[:], scalar1=rr[:],
                                        scalar2=None, op0=OP.mult)
                nc.vector.tensor_tensor(out=yn[:], in0=yn[:], in1=lng_t[:],
                                        op=OP.mult)
                nc.vector.tensor_tensor(out=yn[:], in0=yn[:], in1=lnb_t[:],
                                        op=OP.add)
                nc.sync.dma_start(out=y_out[:], in_=yn[:])

    nc.compile()
    _prog_cache[key] = nc
    return nc


def _wrap16(arr):
    """slot i -> [i % 16, i // 16], replicated into partitions 16..31."""
    n = arr.shape[0]
    out = np.zeros((128, n // 16), np.int16)
    w = arr.reshape(n // 16, 16).T.astype(np.int16)
    out[0:16] = w
    out[16:32] = w
    return out


def _host_prep(inputs, plan):
    node_ids = np.asarray(inputs["node_ids"]).astype(np.int64)
    src = np.asarray(inputs["src"]).astype(np.int64)
    dst = np.asarray(inputs["dst"]).astype(np.int64)
    pad_mask = np.asarray(inputs["pad_mask"])
    ew = np.asarray(inputs["edge_weight"]).astype(np.float64)
    embed = np.asarray(inputs["embed_table"]).astype(np.float64)
    W1 = np.asarray(inputs["W1"]).astype(np.float64)
    b1 = np.asarray(inputs["b1"]).astype(np.float32)
    W2 = np.asarray(inputs["W2"]).astype(np.float32)
    b2 = np.asarray(inputs["b2"]).astype(np.float32)
    W3 = np.asarray(inputs["W3"]).astype(np.float32)
    b3 = np.asarray(inputs["b3"]).astype(np.float32)
    ipw = np.asarray(inputs["in_proj_w"]).astype(np.float64)
    ipb = np.asarray(inputs["in_proj_b"]).astype(np.float64)
    ow = np.asarray(inputs["out_w"]).astype(np.float32)
    ob = np.asarray(inputs["out_b"]).astype(np.float32)
    cls = np.asarray(inputs["cls_embedding"]).astype(np.float64).reshape(D)
    ln_g = np.asarray(inputs["ln_g"]).astype(np.float32)
    ln_b = np.asarray(inputs["ln_b"]).astype(np.float32)

    assert not pad_mask.any(), "kernel compiled for all-False pad_mask"

    T1 = (embed @ W1).astype(BF16)
    Wq, Wk, Wv = ipw[:, :D], ipw[:, D:2 * D], ipw[:, 2 * D:]
    bq, bk_, bv_ = ipb[:D], ipb[D:2 * D], ipb[2 * D:]
    q_cls = (cls @ Wq + bq) / np.sqrt(HD)
    k_cls = cls @ Wk + bk_
    v_cls = cls @ Wv + bv_
    s_cls = np.array([q_cls[h * HD:(h + 1) * HD] @ k_cls[h * HD:(h + 1) * HD]
                      for h in range(H)])
    e_cls = np.exp(s_cls)
    qblk = np.zeros((128, 4), np.float32)
    for h in range(H):
        qblk[h * HD:(h + 1) * HD, h] = q_cls[h * HD:(h + 1) * HD]
    vc4 = np.zeros((4, 128), np.float32)
    for h in range(H):
        vc4[h, h * HD:(h + 1) * HD] = v_cls[h * HD:(h + 1) * HD]
    e4 = np.diag(e_cls).astype(np.float32)
    msel = np.zeros((128, 4), np.float32)
    for h in range(H):
        msel[h * HD:(h + 1) * HD, h] = 1.0
    r4 = np.zeros((4, 128), np.float32)
    for h in range(H):
        r4[h, h * HD:(h + 1) * HD] = 1.0
    w2tiles = np.concatenate(
        [W2[ji * 128:(ji + 1) * 128, jo * 128:(jo + 1) * 128]
         for ji in range(2) for jo in range(2)], axis=1)
    w3tiles = np.concatenate(
        [W3[ji * 128:(ji + 1) * 128, :] for ji in range(2)], axis=1)

    PW = plan['PW']
    iotah_c = np.zeros((128, DH * PW), np.float32)
    for d in range(DH):
        iotah_c[:, d * PW:(d + 1) * PW] = d
    iotal_c = np.zeros((128, DL * PW), np.float32)
    for d in range(DL):
        iotal_c[:, d * PW:(d + 1) * PW] = d

    shared = {
        "t1": T1,
        "b1": b1.astype(np.float32).reshape(2, 128).T.copy(),
        "w2t": w2tiles.astype(BF16),
        "b2": b2.reshape(2, 128).T.copy(),
        "w3t": w3tiles.astype(BF16),
        "b3": b3.reshape(1, 128).T.copy(),
        "iotah": iotah_c.astype(BF16),
        "iotal": iotal_c.astype(BF16),
        "i64": np.vstack([np.eye(64, dtype=np.float32)] * 2).astype(BF16),
        "i128": np.eye(128, dtype=np.float32).astype(BF16),
        "i128f": np.eye(128, dtype=np.float32),
        "i4": np.eye(4, dtype=np.float32).astype(BF16),
        "wk": Wk.astype(np.float32).astype(BF16),
        "bk": bk_.astype(np.float32).reshape(128, 1),
        "wv": Wv.astype(np.float32).astype(BF16),
        "bv": bv_.astype(np.float32).reshape(128, 1),
        "qblk": qblk.astype(BF16),
        "vc4": vc4.astype(BF16),
        "e4": e4.astype(BF16),
        "msel": msel,
        "r4": r4,
        "ecls": e_cls.astype(np.float32).reshape(4, 1),
        "eps": np.full((4, 1), 1e-5, np.float32),
        "ynb": (cls + ob).astype(np.float32).reshape(128, 1),
        "wo": ow.astype(BF16),
        "lng": np.tile(ln_g, (4, 1)),
        "lnb": np.tile(ln_b, (4, 1)),
    }

    ew32 = ew.astype(np.float32)
    smaj = plan['smaj']
    caps = plan['caps']
    slot_off = plan['slot_off']
    SL = plan['SL']
    wpairs = plan['wpairs']
    NPAD = NW * PW
    in_maps = []
    for c in range(NCORES):
        e = plan['eidx'][c]
        sm = smaj[e]
        ch = sm // CH
        starts = np.searchsorted(ch, np.arange(NCH))
        rank = np.arange(len(e)) - starts[ch]
        slots = slot_off[ch] + rank
        g_idx = np.zeros(SL, np.int64)
        sl_dst = np.full(SL, DSENT, np.float32)
        sl_ew = np.zeros(SL, np.float32)
        g_idx[slots] = src[e] - c * NPC
        sl_dst[slots] = (sm % CH).astype(np.float32)
        sl_ew[slots] = ew32[e]
        dstl_pad = np.full((128, NPAD), DSENT, np.float32)
        eww_pad = np.zeros((128, NPAD), np.float32)
        for w in range(NW):
            for jl, (bl, k) in enumerate(wpairs[w]):
                blo = (plan['wbase'][w] // 128 + bl) * 128
                klo, khi = slot_off[k], slot_off[k] + caps[k]
                lo = max(blo, klo) - blo
                hi = min(blo + 128, khi) - blo
                col = w * PW + jl
                dstl_pad[lo:hi, col] = sl_dst[blo + lo:blo + hi]
                eww_pad[lo:hi, col] = sl_ew[blo + lo:blo + hi]

        nids_own = node_ids[c * NPC:(c + 1) * NPC]
        ids_e_mask = (src // NPC) == c
        # layer-0 weighted count matrix over vocab for OWN dsts
        own_dst_mask = (dst // NPC) == c
        ids_e = node_ids[src[own_dst_mask]]
        dl_e = dst[own_dst_mask] - c * NPC
        Cf = np.bincount(dl_e * N + ids_e, weights=ew[own_dst_mask],
                         minlength=NPC * N).reshape(NPC, N).astype(np.float32)
        CtT = Cf.T.astype(BF16)
        ct_tiles = CtT.reshape(N // 128, 128, NPC).transpose(1, 0, 2).copy()
        m = dict(shared)
        dsth_pad = np.floor(dstl_pad / DL)
        dstl8_pad = dstl_pad - dsth_pad * DL
        m.update({
            "h0w1own": (T1.astype(np.float32)[nids_own]
                        + b1[None, :].astype(np.float32)).astype(BF16),
            "ct": ct_tiles,
            "idx12": _wrap16(g_idx),
            "dsth": dsth_pad.astype(BF16),
            "dstl8": dstl8_pad.astype(BF16),
            "eww": eww_pad.astype(BF16),
        })
        in_maps.append(m)
    return in_maps


def kernel(**inputs):
    from concourse.bass_utils import run_bass_kernel_spmd
    plan = _build_plan(inputs["src"], inputs["dst"])
    nc = _build_program(plan)
    in_maps = _host_prep(inputs, plan)
    res = run_bass_kernel_spmd(nc, in_maps, core_ids=list(range(NCORES)))
    y = np.concatenate([res.results[c]["y"] for c in range(NCORES)], axis=0)
    return np.ascontiguousarray(y.astype(np.float32))


# revision 15
# speedup vs baseline: 1.1105x; 1.1105x over previous
"""Trainium2 Bass kernel v2 for nn_GTShapelet (GIN stack + CLS-query MHA).

Structure change vs v1: instead of AllGather-ing full h between GIN layers
(2 x 265us collectives), messages are computed source-sharded and combined
with ReduceScatter (output 1/8 the size -> ~4x cheaper collective):

  - Each core owns nodes [c*4096, (c+1)*4096) and the edges whose SOURCE
    lies in its range.
  - Layer algebra: (h + segsum(ew*h[src])) @ W + b
      = g_own + segsum(ew*g[src]) + b', where g := h @ W (+bias folded).
    So each layer's exchange carries the PRE-PROJECTED g rows; after the
    ReduceScatter only an elementwise add + gelu remains.
  - Per layer: gather own-edge source rows from the core's own g table in
    HBM (512B/desc), selection-matmul them into 128-dst-chunk psums
    (contiguous slot packing, boundary-spanning batches), write bf16
    partials [32768, F] stripe-major, ReduceScatter(add) in 4 stripes
    overlapped with compute.
  - L1 stays the dense vocab-matmul (ct @ T1) since node ids live in a
    1024-row table; W2 is fused right after per stripe.
"""

import sys

if "/opt/trn_rl_repo" not in sys.path:
    sys.path.insert(0, "/opt/trn_rl_repo")

import numpy as np
import ml_dtypes  # noqa: F401

# ---- problem constants (hardcoded per spec) ----
B, N, E, D = 32, 1024, 524288, 128
H, HD = 4, 32
F2 = 2 * D                      # 256
NCORES = 8
NPC = B * N // NCORES           # 4096 own nodes per core
GPC = B // NCORES               # 4 graphs per core
CH = 128                        # dst chunk width (smaj positions)
NCH = B * N // CH               # 256 chunks globally
WCH = 8                         # chunks per gather window
NW = NCH // WCH                 # 16 windows
NSTR = 4                        # ReduceScatter stripes
OWN_STR = NPC // NSTR           # 1024 own rows per stripe
STR_ROWS = B * N // NSTR        # 8192 smaj rows per stripe
DH, DL = 16, 8                  # chunk one-hot factorization: d = 8*dh + dl
DSENT = 200.0
BF16 = np.dtype('bfloat16')
FP8 = np.dtype(ml_dtypes.float8_e4m3)

_prog_cache = {}


def _smaj_of(d):
    return ((d % NPC) // OWN_STR * (NCORES * OWN_STR)
            + (d // NPC) * OWN_STR + (d % OWN_STR))


def _build_plan(src, dst):
    """Static slot/pair plan shared by all cores (max-over-cores caps)."""
    src = np.asarray(src).astype(np.int64)
    dst = np.asarray(dst).astype(np.int64)
    smaj = _smaj_of(dst)
    core_of = src // NPC
    counts = np.zeros((NCORES, NCH), np.int64)
    eidx = []
    for c in range(NCORES):
        m = np.nonzero(core_of == c)[0]
        e = m[np.argsort(smaj[m], kind='stable')]
        eidx.append(e)
        counts[c] = np.bincount(smaj[e] // CH, minlength=NCH)
    caps = counts.max(axis=0)
    slot_off = np.zeros(NCH + 1, np.int64)
    wcap = np.zeros(NW, np.int64)
    wbase = np.zeros(NW, np.int64)
    off = 0
    for w in range(NW):
        wbase[w] = off
        for k in range(w * WCH, (w + 1) * WCH):
            slot_off[k] = off
            off += caps[k]
        off = (off + 127) // 128 * 128
        wcap[w] = off - wbase[w]
    slot_off[NCH] = off
    SL = off
    # pairs: (w, batch, chunk); per-window local enumeration
    wpairs = []          # per window: list of (b_local, k)
    for w in range(NW):
        pw = []
        for b in range(wbase[w] // 128, (wbase[w] + wcap[w]) // 128):
            blo, bhi = b * 128, b * 128 + 128
            for k in range(w * WCH, (w + 1) * WCH):
                klo, khi = slot_off[k], slot_off[k] + caps[k]
                if klo < bhi and blo < khi:
                    pw.append((b - wbase[w] // 128, k))
        wpairs.append(pw)
    PW = max(len(pw) for pw in wpairs)
    return dict(caps=caps, slot_off=slot_off, wcap=wcap, wbase=wbase,
                SL=SL, wpairs=wpairs, PW=PW, eidx=eidx, smaj=smaj)


def _build_program(plan, variant="hw"):
    key = (variant, plan['SL'], plan['PW'])
    if key in _prog_cache:
        return _prog_cache[key]
    import concourse.bacc as bacc
    import concourse.tile as tile
    import concourse.mybir as mybir
    from concourse.library_config import mlp

    dt = mybir.dt
    AF = mybir.ActivationFunctionType
    OP = mybir.AluOpType
    AF_GELU = AF.Identity if variant == "sim" else AF.Gelu

    SL = int(plan['SL'])
    PW = int(plan['PW'])
    NPAD = NW * PW
    wcap = [int(x) for x in plan['wcap']]
    wbase = [int(x) for x in plan['wbase']]
    wpairs = plan['wpairs']
    NBMAX = max(wcap) // 128

    nc = bacc.Bacc("TRN2", target_bir_lowering=False, debug=False,
                   num_devices=NCORES)

    def din(name, shape, dtype):
        return nc.dram_tensor(name, shape, dtype, kind="ExternalInput")

    t1 = din("t1", [N, F2], dt.float8e4)
    h0w1own = din("h0w1own", [NPC, F2], dt.bfloat16)
    ct = din("ct", [128, N // 128, NPC], dt.float8e4)
    b1 = din("b1", [128, 2], dt.float32)
    w2t = din("w2t", [128, 4 * 128], dt.bfloat16)
    b2 = din("b2", [128, 2], dt.float32)
    w3t = din("w3t", [128, 2 * 128], dt.bfloat16)
    b3 = din("b3", [128, 1], dt.float32)
    idx12 = din("idx12", [128, SL // 16], dt.int16)
    dsth = din("dsth", [128, NPAD], dt.bfloat16)
    dstl8 = din("dstl8", [128, NPAD], dt.bfloat16)
    eww = din("eww", [128, NPAD], dt.bfloat16)
    iotah = din("iotah", [128, DH * PW], dt.bfloat16)
    iotal = din("iotal", [128, DL * PW], dt.bfloat16)
    i64 = din("i64", [128, 64], dt.bfloat16)
    i128 = din("i128", [128, 128], dt.bfloat16)
    i128f = din("i128f", [128, 128], dt.float32)
    i4 = din("i4", [4, 4], dt.bfloat16)
    wk = din("wk", [128, 128], dt.bfloat16)
    bk = din("bk", [128, 1], dt.float32)
    wv = din("wv", [128, 128], dt.bfloat16)
    bv = din("bv", [128, 1], dt.float32)
    qblk = din("qblk", [128, 4], dt.bfloat16)
    vc4 = din("vc4", [4, 128], dt.bfloat16)
    e4 = din("e4", [4, 4], dt.bfloat16)
    msel = din("msel", [128, 4], dt.float32)
    r4 = din("r4", [4, 128], dt.float32)
    ecls = din("ecls", [4, 1], dt.float32)
    eps = din("eps", [4, 1], dt.float32)
    ynb = din("ynb", [128, 1], dt.float32)
    wo = din("wo", [128, 128], dt.bfloat16)
    lng = din("lng", [4, 128], dt.float32)
    lnb = din("lnb", [4, 128], dt.float32)
    y_out = nc.dram_tensor("y", [GPC, D], dt.float32, kind="ExternalOutput")

    with tile.TileContext(nc) as tc:
        nc.gpsimd.load_library(mlp)
        with tc.tile_pool(name="const", bufs=1) as cp, \
             tc.tile_pool(name="res", bufs=1) as rp, \
             tc.tile_pool(name="work", bufs=1) as wp, \
             tc.tile_pool(name="dram", bufs=1, space="DRAM") as dram:

            def cload(ap, shape, dtype):
                t = cp.tile(shape, dtype, name=f"c_{ap.name}")
                nc.sync.dma_start(out=t[:], in_=ap[:])
                return t

            b1_t = cload(b1, [128, 2], dt.float32)
            w2t_t = cload(w2t, [128, 4 * 128], dt.bfloat16)
            b2_t = cload(b2, [128, 2], dt.float32)
            w3t_t = cload(w3t, [128, 2 * 128], dt.bfloat16)
            b3_t = cload(b3, [128, 1], dt.float32)
            i64_t = cload(i64, [128, 64], dt.bfloat16)
            i128_t = cload(i128, [128, 128], dt.bfloat16)
            i128f_t = cload(i128f, [128, 128], dt.float32)
            i4_t = cload(i4, [4, 4], dt.bfloat16)
            wk_t = cload(wk, [128, 128], dt.bfloat16)
            bk_t = cload(bk, [128, 1], dt.float32)
            wv_t = cload(wv, [128, 128], dt.bfloat16)
            bv_t = cload(bv, [128, 1], dt.float32)
            qblk_t = cload(qblk, [128, 4], dt.bfloat16)
            vc4_t = cload(vc4, [4, 128], dt.bfloat16)
            e4_t = cload(e4, [4, 4], dt.bfloat16)
            msel_t = cload(msel, [128, 4], dt.float32)
            r4_t = cload(r4, [4, 128], dt.float32)
            ecls_t = cload(ecls, [4, 1], dt.float32)
            eps_t = cload(eps, [4, 1], dt.float32)
            ynb_t = cload(ynb, [128, 1], dt.float32)
            wo_t = cload(wo, [128, 128], dt.bfloat16)
            lng_t = cload(lng, [4, 128], dt.float32)
            lnb_t = cload(lnb, [4, 128], dt.float32)

            # persistent SBUF (feature-major g tables for post-RS adds)
            g1T = rp.tile([128, 2, NPC], dt.bfloat16, name="g1T")
            g2T = rp.tile([128, NPC], dt.bfloat16, name="g2T")
            h3T = rp.tile([128, NPC], dt.bfloat16, name="h3T")

            # DRAM scratch
            g1own = dram.tile([NPC, F2], dt.bfloat16, tag="g1own")
            g2own = dram.tile([NPC, D], dt.bfloat16, tag="g2own")
            part2 = dram.tile([B * N, F2], dt.bfloat16, tag="part2")
            msg2 = dram.tile([NPC, F2], dt.bfloat16, tag="msg2")
            part3 = dram.tile([B * N, D], dt.bfloat16, tag="part3")
            msg3 = dram.tile([NPC, D], dt.bfloat16, tag="msg3")

            g1own_v = g1own.rearrange("(t p) f -> p t f", p=128)
            g2own_v = g2own.rearrange("(t p) f -> p t f", p=128)

            # ================= L1 + fused W2 =================
            with tc.tile_pool(name="l1_ps", bufs=1, space="PSUM") as pp, \
                 tc.tile_pool(name="l1_sb", bufs=1) as gp:
                t1sb = gp.tile([128, N // 128, F2], dt.float8e4, name="t1sb")
                nc.sync.dma_start(
                    out=t1sb[:],
                    in_=t1.rearrange("(kk p) f -> p kk f", p=128))
                hown0 = gp.tile([128, NPC // 128, F2], dt.bfloat16,
                                name="hown0")
                nc.sync.dma_start(
                    out=hown0[:],
                    in_=h0w1own.rearrange("(t p) f -> p t f", p=128))
                hT1 = gp.tile([128, 2, NPC], dt.bfloat16, name="hT1")
                ctbufs = [gp.tile([128, N // 128, 512], dt.float8e4,
                                  name=f"ctb{i}") for i in range(2)]
                def w2_phase(s):
                    # W2 for the 512 nodes of ct-segment s (chunks 8s..8s+8)
                    nsl = slice(s * 512, (s + 1) * 512)
                    for jo in range(2):
                        ps2 = pp.tile([128, 512], dt.float32, tag="w2p",
                                      bufs=2)
                        for ji in range(2):
                            nc.tensor.matmul(
                                out=ps2[:],
                                lhsT=w2t_t[:, (2 * ji + jo) * 128:
                                           (2 * ji + jo + 1) * 128],
                                rhs=hT1[:, ji, nsl],
                                start=(ji == 0), stop=(ji == 1))
                        nc.scalar.activation(
                            g1T[:, jo, nsl], ps2[:],
                            AF.Identity, bias=b2_t[:, jo:jo + 1])
                    g1st = gp.tile([128, 4, F2], dt.bfloat16, tag="g1st",
                                   bufs=2)
                    for t in range(4):
                        for j in range(2):
                            tp2 = pp.tile([128, 128], dt.bfloat16, tag="tp2",
                                          bufs=2)
                            nc.tensor.transpose(
                                tp2[:],
                                g1T[:, j, s * 512 + t * 128:
                                    s * 512 + (t + 1) * 128],
                                i128_t[:])
                            nc.vector.tensor_copy(
                                out=g1st[:, t, j * 128:(j + 1) * 128],
                                in_=tp2[:])
                    nc.sync.dma_start(
                        out=g1own_v[:, s * 4:(s + 1) * 4, :], in_=g1st[:])

                for s in range(8):
                    ctb = ctbufs[s % 2]
                    nc.sync.dma_start(
                        out=ctb[:], in_=ct[:, :, s * 512:(s + 1) * 512])
                    for kk in range(4):
                        # 128-dst chunks: half the activation/transpose calls
                        # of the 64-dst layout (ACT cost is per-column)
                        k = s * 4 + kk
                        ps = pp.tile([128, F2], dt.float32, tag="l1p",
                                     bufs=2)
                        for kt in range(N // 128):
                            nc.tensor.matmul(
                                out=ps[:],
                                lhsT=ctb[:, kt, kk * 128:(kk + 1) * 128],
                                rhs=t1sb[:, kt, :],
                                start=(kt == 0), stop=False)
                        nc.tensor.matmul(
                            out=ps[:], lhsT=i128_t[:],
                            rhs=hown0[:, k, :],
                            start=False, stop=True)
                        # b1 is folded into h0w1own on the host, so the
                        # psum drain IS the gelu
                        msb = gp.tile([128, F2], dt.bfloat16, tag="msb",
                                      bufs=3)
                        nc.scalar.activation(msb[:], ps[:], AF_GELU)
                        for j in range(2):
                            tp = pp.tile([128, 128], dt.bfloat16, tag="tp",
                                         bufs=2)
                            nc.tensor.transpose(
                                tp[:], msb[:, j * 128:(j + 1) * 128],
                                i128_t[:])
                            nc.vector.tensor_copy(
                                out=hT1[:, j, k * 128:(k + 1) * 128],
                                in_=tp[:])
                    # issue previous segment's W2 phase behind this one's
                    # chunks so its latency chain hides under ct matmuls
                    if s >= 1:
                        w2_phase(s - 1)
                w2_phase(7)

            # exchange consts load late so they overlap L1 compute
            idx12_t = cload(idx12, [128, SL // 16], dt.int16)
            dsth_t = cload(dsth, [128, NPAD], dt.bfloat16)
            dstl8_t = cload(dstl8, [128, NPAD], dt.bfloat16)
            eww_t = cload(eww, [128, NPAD], dt.bfloat16)
            iotah_t = cload(iotah, [128, DH * PW], dt.bfloat16)
            iotal_t = cload(iotal, [128, DL * PW], dt.bfloat16)

            # attention state shared across L3 stripes
            kT = rp.tile([128, NPC], dt.bfloat16, name="kT")
            vnm = rp.tile([128, NPC // 128, 128], dt.bfloat16, name="vnm")
            ctx_all = rp.tile([128, 4], dt.bfloat16, name="ctx_all")

            # ============ exchange layers (L2: F2-wide, L3: D-wide) ============
            def exchange_layer(F, gsrc, part, msg, post_stripe, pwait):
                with tc.tile_pool(name="ex_ps", bufs=1, space="PSUM") as pp, \
                     tc.tile_pool(name="ex_sb", bufs=1) as xp:
                    gbufs = [xp.tile([128, NBMAX, F], dt.bfloat16,
                                     name=f"gb{i}") for i in range(3)]
                    svbufs = [xp.tile([128, CH * PW], dt.bfloat16,
                                      name=f"sv{i}") for i in range(3)]
                    stbufs = [xp.tile([128, WCH, F], dt.bfloat16,
                                      name=f"st{i}") for i in range(2)]
                    part_v = part.rearrange("(w q p) f -> p w q f", p=128,
                                            w=NW)

                    def issue_rs(sidx):
                        nc.gpsimd.collective_compute(
                            "ReduceScatter", OP.add,
                            replica_groups=[list(range(NCORES))],
                            ins=[part[sidx * STR_ROWS:
                                      (sidx + 1) * STR_ROWS, :].opt()],
                            outs=[msg[sidx * OWN_STR:
                                      (sidx + 1) * OWN_STR, :].opt()])

                    for w in range(NW):
                        nb = wcap[w] // 128
                        gb = gbufs[w % 3]
                        # sub-calls of <=1024 descriptors: the SWDGE ring
                        # holds 1024 entries, larger single calls overflow it
                        for b0 in range(0, nb, 8):
                            nsub = min(8, nb - b0) * 128
                            base = wbase[w] + b0 * 128
                            nc.gpsimd.dma_gather(
                                gb[:, b0:b0 + nsub // 128, :], gsrc[:],
                                idx12_t[:, base // 16:(base + nsub) // 16],
                                nsub, nsub, F)
                        # S = onehot(dsth) x (onehot(dstl8) * ew), d=8*dh+dl
                        svh = xp.tile([128, DH, PW], dt.bfloat16, tag="svh",
                                      bufs=2)
                        svl = xp.tile([128, DL, PW], dt.bfloat16, tag="svl",
                                      bufs=2)
                        nc.vector.tensor_tensor(
                            out=svh[:],
                            in0=dsth_t[:, w * PW:(w + 1) * PW].unsqueeze(1)
                                .broadcast_to([128, DH, PW]),
                            in1=iotah_t[:].rearrange("p (dh j) -> p dh j",
                                                     j=PW),
                            op=OP.is_equal)
                        nc.vector.tensor_tensor(
                            out=svl[:],
                            in0=dstl8_t[:, w * PW:(w + 1) * PW].unsqueeze(1)
                                .broadcast_to([128, DL, PW]),
                            in1=iotal_t[:].rearrange("p (dl j) -> p dl j",
                                                     j=PW),
                            op=OP.is_equal)
                        nc.vector.tensor_tensor(
                            out=svl[:], in0=svl[:],
                            in1=eww_t[:, w * PW:(w + 1) * PW].unsqueeze(1)
                                .broadcast_to([128, DL, PW]),
                            op=OP.mult)
                        sv = svbufs[w % 3]
                        nc.vector.tensor_tensor(
                            out=sv[:].rearrange("p (dh dl j) -> p dh dl j",
                                                dl=DL, j=PW),
                            in0=svh[:].unsqueeze(2)
                                .broadcast_to([128, DH, DL, PW]),
                            in1=svl[:].unsqueeze(1)
                                .broadcast_to([128, DH, DL, PW]),
                            op=OP.mult)
                        svv = sv[:].rearrange("p (d j) -> p d j", j=PW)
                        st = stbufs[w % 2]
                        for kk2 in range(WCH // 2):
                            ps = pp.tile([128, 2 * F], dt.float32, tag="selp",
                                         bufs=2)
                            for half in range(2):
                                kk = kk2 * 2 + half
                                k = w * WCH + kk
                                prs = [(jl, bl) for jl, (bl, kq) in
                                       enumerate(wpairs[w]) if kq == k]
                                for i, (jl, bl) in enumerate(prs):
                                    nc.tensor.matmul(
                                        out=ps[:, half * F:(half + 1) * F],
                                        lhsT=svv[:, :, jl],
                                        rhs=gb[:, bl, :],
                                        start=(i == 0),
                                        stop=(i == len(prs) - 1))
                            psv3 = ps[:].rearrange("p (q f) -> p q f", q=2)
                            if kk2 % 2 == 0:
                                nc.scalar.activation(
                                    st[:, kk2 * 2:kk2 * 2 + 2, :], psv3,
                                    AF.Copy)
                            else:
                                nc.vector.tensor_copy(
                                    out=st[:, kk2 * 2:kk2 * 2 + 2, :],
                                    in_=psv3)
                        nc.sync.dma_start(out=part_v[:, w, :, :], in_=st[:])
                        # deferred issue: RS for stripe s-1 goes out 2 windows
                        # into stripe s (its pwrite wait is then ~done); the
                        # post-work for stripe s-2 goes out mid stripe s (its
                        # RS finished during stripe s-1) - so no instruction
                        # ever stalls an in-order engine queue that feeds the
                        # gather/matmul pipeline
                        wps = NW // NSTR
                        if w % wps == 2 and w > wps:
                            issue_rs(w // wps - 1)
                        if w % wps == 4 and w > 2 * wps:
                            sp = w // wps - 2
                            with tc.tile_wait_until(
                                    pwait[0] + pwait[1] * sp,
                                    enable=pwait[0] > 0):
                                post_stripe(sp, pp, xp)
                    issue_rs(NSTR - 1)
                    for sp in (NSTR - 2, NSTR - 1):
                        with tc.tile_wait_until(pwait[0] + pwait[1] * sp,
                                                enable=pwait[0] > 0):
                            post_stripe(sp, pp, xp)

            def post_stripe_l2(s, pp, xp):
                # msg readback lands feature-major via XBAR transpose-DMA
                u2T = xp.tile([128, 2, OWN_STR], dt.bfloat16, tag="u2T",
                              bufs=1)
                for j in range(2):
                    nc.sync.dma_start_transpose(
                        u2T[:, j, :],
                        msg2[s * OWN_STR:(s + 1) * OWN_STR,
                             j * 128:(j + 1) * 128])
                nsl = slice(s * OWN_STR, (s + 1) * OWN_STR)
                z2T = xp.tile([128, 2, OWN_STR], dt.bfloat16, tag="z2T",
                              bufs=1)
                nc.vector.tensor_tensor(out=z2T[:], in0=u2T[:],
                                        in1=g1T[:, :, nsl], op=OP.add)
                h2T_st = xp.tile([128, 2, OWN_STR], dt.bfloat16, tag="h2T",
                                 bufs=1)
                nc.scalar.activation(h2T_st[:], z2T[:], AF_GELU)
                for m in range(2):
                    ps3 = pp.tile([128, 512], dt.float32, tag="w3p", bufs=2)
                    for ji in range(2):
                        nc.tensor.matmul(
                            out=ps3[:],
                            lhsT=w3t_t[:, ji * 128:(ji + 1) * 128],
                            rhs=h2T_st[:, ji, m * 512:(m + 1) * 512],
                            start=(ji == 0), stop=(ji == 1))
                    nc.scalar.activation(
                        g2T[:, s * OWN_STR + m * 512:
                            s * OWN_STR + (m + 1) * 512],
                        ps3[:], AF.Identity, bias=b3_t[:, 0:1])
                g2st = xp.tile([128, 8, D], dt.bfloat16, tag="g2st", bufs=2)
                for t in range(8):
                    tp = pp.tile([128, 128], dt.bfloat16, tag="xtp", bufs=2)
                    nc.tensor.transpose(
                        tp[:],
                        g2T[:, s * OWN_STR + t * 128:
                            s * OWN_STR + (t + 1) * 128],
                        i128_t[:])
                    if t % 2 == 0:
                        nc.vector.tensor_copy(out=g2st[:, t, :], in_=tp[:])
                    else:
                        nc.scalar.activation(g2st[:, t, :], tp[:], AF.Copy)
                nc.sync.dma_start(out=g2own_v[:, s * 8:(s + 1) * 8, :],
                                  in_=g2st[:])

            def post_stripe_l3(s, pp, xp):
                u3T = xp.tile([128, OWN_STR], dt.bfloat16, tag="u3T", bufs=1)
                nc.sync.dma_start_transpose(
                    u3T[:], msg3[s * OWN_STR:(s + 1) * OWN_STR, :])
                nsl = slice(s * OWN_STR, (s + 1) * OWN_STR)
                z3T = xp.tile([128, OWN_STR], dt.bfloat16, tag="z3T", bufs=1)
                nc.vector.tensor_tensor(out=z3T[:], in0=u3T[:],
                                        in1=g2T[:, nsl], op=OP.add)
                nc.scalar.activation(h3T[:, nsl], z3T[:], AF_GELU)
                # attention for graph g == stripe s (graphs are 1024 nodes)
                g = s
                for m2 in range(2):
                    nsl = slice(s * 1024 + m2 * 512, s * 1024 + (m2 + 1) * 512)
                    psk = pp.tile([128, 512], dt.float32, tag="psk", bufs=1)
                    nc.tensor.matmul(out=psk[:], lhsT=wk_t[:], rhs=h3T[:, nsl])
                    nc.vector.tensor_scalar(
                        out=kT[:, nsl], in0=psk[:],
                        scalar1=bk_t[:], scalar2=None, op0=OP.add)
                for t in range(8):
                    psv = pp.tile([128, 128], dt.float32, tag="psv", bufs=1)
                    nc.tensor.matmul(
                        out=psv[:],
                        lhsT=h3T[:, (s * 8 + t) * 128:(s * 8 + t + 1) * 128],
                        rhs=wv_t[:])
                    nc.scalar.activation(vnm[:, s * 8 + t, :], psv[:], AF.Copy)
                expt = xp.tile([4, 1024], dt.bfloat16, tag="expt", bufs=1)
                sums2 = xp.tile([4, 2], dt.float32, tag="sums2", bufs=1)
                for hh in range(2):
                    ssc = pp.tile([4, 512], dt.float32, tag="ssc", bufs=1)
                    nc.tensor.matmul(
                        out=ssc[:], lhsT=qblk_t[:],
                        rhs=kT[:, g * 1024 + hh * 512:
                               g * 1024 + (hh + 1) * 512])
                    nc.scalar.activation(expt[:, hh * 512:(hh + 1) * 512],
                                         ssc[:], AF.Exp,
                                         accum_out=sums2[:, hh:hh + 1])
                sums = xp.tile([4, 1], dt.float32, tag="sums", bufs=1)
                nc.vector.tensor_add(out=sums[:], in0=sums2[:, 0:1],
                                     in1=sums2[:, 1:2])
                nc.vector.tensor_add(out=sums[:], in0=sums[:], in1=ecls_t[:])
                psr = pp.tile([128, 1], dt.float32, tag="ptiny", bufs=1)
                nc.tensor.matmul(out=psr[:], lhsT=r4_t[:], rhs=sums[:])
                rbc = xp.tile([128, 1], dt.float32, tag="rbc", bufs=1)
                nc.vector.reciprocal(rbc[:], psr[:])
                psctx = pp.tile([128, 4], dt.float32, tag="psctx", bufs=1)
                for t in range(8):
                    pst = pp.tile([128, 4], dt.bfloat16, tag="ptiny", bufs=1)
                    nc.tensor.transpose(
                        pst[:], expt[:, t * 128:(t + 1) * 128], i4_t[:])
                    ets = xp.tile([128, 4], dt.bfloat16, tag="ets", bufs=1)
                    nc.vector.tensor_copy(out=ets[:], in_=pst[:])
                    nc.tensor.matmul(out=psctx[:],
                                     lhsT=vnm[:, g * 8 + t, :], rhs=ets[:],
                                     start=(t == 0), stop=False)
                nc.tensor.matmul(out=psctx[:], lhsT=vc4_t[:], rhs=e4_t[:],
                                 start=False, stop=True)
                tmp4 = xp.tile([128, 4], dt.float32, tag="tmp4", bufs=1)
                nc.vector.tensor_tensor(out=tmp4[:], in0=psctx[:],
                                        in1=msel_t[:], op=OP.mult)
                ctxv = xp.tile([128, 1], dt.float32, tag="ctxv", bufs=1)
                nc.vector.reduce_sum(out=ctxv[:], in_=tmp4[:],
                                     axis=mybir.AxisListType.X)
                nc.vector.tensor_scalar(out=ctxv[:], in0=ctxv[:],
                                        scalar1=rbc[:], scalar2=bv_t[:],
                                        op0=OP.mult, op1=OP.add)
                nc.vector.tensor_copy(out=ctx_all[:, g:g + 1], in_=ctxv[:])

            exchange_layer(F2, g1own, part2, msg2, post_stripe_l2,
                           (0.197, 0.038))
            exchange_layer(D, g2own, part3, msg3, post_stripe_l3,
                           (0.41, 0.037))

            # ============ final out-projection + layernorm ============
            with tc.tile_pool(name="att_ps", bufs=1, space="PSUM") as ap_, \
                 tc.tile_pool(name="att_sb", bufs=1) as asb:
                psao = ap_.tile([128, 4], dt.float32, tag="ptiny", bufs=2)
                nc.tensor.matmul(out=psao[:], lhsT=wo_t[:], rhs=ctx_all[:])
                ysb = asb.tile([128, 4], dt.float32, tag="ysb")
                nc.vector.tensor_scalar(out=ysb[:], in0=psao[:],
                                        scalar1=ynb_t[:], scalar2=None,
                                        op0=OP.add)
                psy = ap_.tile([4, 128], dt.float32, tag="ptiny", bufs=2)
                nc.tensor.matmul(out=psy[:], lhsT=ysb[:], rhs=i128f_t[:],
                                 is_transpose=True)
                yt = asb.tile([4, 128], dt.float32, tag="yt")
                nc.vector.tensor_copy(out=yt[:], in_=psy[:])
                mn = asb.tile([4, 1], dt.float32, tag="mn")
                nc.vector.reduce_sum(out=mn[:], in_=yt[:],
                                     axis=mybir.AxisListType.X)
                nc.vector.tensor_scalar(out=mn[:], in0=mn[:],
                                        scalar1=1.0 / D, scalar2=None,
                                        op0=OP.mult)
                xc = asb.tile([4, 128], dt.float32, tag="xc")
                nc.vector.tensor_scalar(out=xc[:], in0=yt[:], scalar1=mn[:],
                                        scalar2=None, op0=OP.subtract)
                sq = asb.tile([4, 128], dt.float32, tag="sq")
                ss = asb.tile([4, 1], dt.float32, tag="ss")
                nc.scalar.activation(sq[:], xc[:], AF.Square, accum_out=ss[:])
                sd = asb.tile([4, 1], dt.float32, tag="sd")
                nc.scalar.activation(sd[:], ss[:], AF.Sqrt, bias=eps_t[:],
                                     scale=1.0 / D)
                rr = asb.tile([4, 1], dt.float32, tag="rr")
                nc.vector.reciprocal(rr[:], sd[:])
                yn = asb.tile([4, 128], dt.float32, tag="yn")
                nc.vector.tensor_scalar(out=yn[:], in0=xc[:], scalar1=rr[:],
                                        scalar2=None, op0=OP.mult)
                nc.vector.tensor_tensor(out=yn[:], in0=yn[:], in1=lng_t[:],
                                        op=OP.mult)
                nc.vector.tensor_tensor(out=yn[:], in0=yn[:], in1=lnb_t[:],
                                        op=OP.add)
                nc.sync.dma_start(out=y_out[:], in_=yn[:])

    nc.compile()
    _prog_cache[key] = nc
    return nc


def _wrap16(arr):
    """slot i -> [i % 16, i // 16], replicated into partitions 16..31."""
    n = arr.shape[0]
    out = np.zeros((128, n // 16), np.int16)
    w = arr.reshape(n // 16, 16).T.astype(np.int16)
    out[0:16] = w
    out[16:32] = w
    return out


def _host_prep(inputs, plan):
    node_ids = np.asarray(inputs["node_ids"]).astype(np.int64)
    src = np.asarray(inputs["src"]).astype(np.int64)
    dst = np.asarray(inputs["dst"]).astype(np.int64)
    pad_mask = np.asarray(inputs["pad_mask"])
    ew = np.asarray(inputs["edge_weight"]).astype(np.float64)
    embed = np.asarray(inputs["embed_table"]).astype(np.float64)
    W1 = np.asarray(inputs["W1"]).astype(np.float64)
    b1 = np.asarray(inputs["b1"]).astype(np.float32)
    W2 = np.asarray(inputs["W2"]).astype(np.float32)
    b2 = np.asarray(inputs["b2"]).astype(np.float32)
    W3 = np.asarray(inputs["W3"]).astype(np.float32)
    b3 = np.asarray(inputs["b3"]).astype(np.float32)
    ipw = np.asarray(inputs["in_proj_w"]).astype(np.float64)
    ipb = np.asarray(inputs["in_proj_b"]).astype(np.float64)
    ow = np.asarray(inputs["out_w"]).astype(np.float32)
    ob = np.asarray(inputs["out_b"]).astype(np.float32)
    cls = np.asarray(inputs["cls_embedding"]).astype(np.float64).reshape(D)
    ln_g = np.asarray(inputs["ln_g"]).astype(np.float32)
    ln_b = np.asarray(inputs["ln_b"]).astype(np.float32)

    assert not pad_mask.any(), "kernel compiled for all-False pad_mask"

    T1 = (embed @ W1).astype(BF16)
    Wq, Wk, Wv = ipw[:, :D], ipw[:, D:2 * D], ipw[:, 2 * D:]
    bq, bk_, bv_ = ipb[:D], ipb[D:2 * D], ipb[2 * D:]
    q_cls = (cls @ Wq + bq) / np.sqrt(HD)
    k_cls = cls @ Wk + bk_
    v_cls = cls @ Wv + bv_
    s_cls = np.array([q_cls[h * HD:(h + 1) * HD] @ k_cls[h * HD:(h + 1) * HD]
                      for h in range(H)])
    e_cls = np.exp(s_cls)
    qblk = np.zeros((128, 4), np.float32)
    for h in range(H):
        qblk[h * HD:(h + 1) * HD, h] = q_cls[h * HD:(h + 1) * HD]
    vc4 = np.zeros((4, 128), np.float32)
    for h in range(H):
        vc4[h, h * HD:(h + 1) * HD] = v_cls[h * HD:(h + 1) * HD]
    e4 = np.diag(e_cls).astype(np.float32)
    msel = np.zeros((128, 4), np.float32)
    for h in range(H):
        msel[h * HD:(h + 1) * HD, h] = 1.0
    r4 = np.zeros((4, 128), np.float32)
    for h in range(H):
        r4[h, h * HD:(h + 1) * HD] = 1.0
    w2tiles = np.concatenate(
        [W2[ji * 128:(ji + 1) * 128, jo * 128:(jo + 1) * 128]
         for ji in range(2) for jo in range(2)], axis=1)
    w3tiles = np.concatenate(
        [W3[ji * 128:(ji + 1) * 128, :] for ji in range(2)], axis=1)

    PW = plan['PW']
    iotah_c = np.zeros((128, DH * PW), np.float32)
    for d in range(DH):
        iotah_c[:, d * PW:(d + 1) * PW] = d
    iotal_c = np.zeros((128, DL * PW), np.float32)
    for d in range(DL):
        iotal_c[:, d * PW:(d + 1) * PW] = d

    shared = {
        "t1": T1.astype(np.float32).astype(FP8),
        "b1": b1.astype(np.float32).reshape(2, 128).T.copy(),
        "w2t": w2tiles.astype(BF16),
        "b2": b2.reshape(2, 128).T.copy(),
        "w3t": w3tiles.astype(BF16),
        "b3": b3.reshape(1, 128).T.copy(),
        "iotah": iotah_c.astype(BF16),
        "iotal": iotal_c.astype(BF16),
        "i64": np.vstack([np.eye(64, dtype=np.float32)] * 2).astype(BF16),
        "i128": np.eye(128, dtype=np.float32).astype(BF16),
        "i128f": np.eye(128, dtype=np.float32),
        "i4": np.eye(4, dtype=np.float32).astype(BF16),
        "wk": Wk.astype(np.float32).astype(BF16),
        "bk": bk_.astype(np.float32).reshape(128, 1),
        "wv": Wv.astype(np.float32).astype(BF16),
        "bv": bv_.astype(np.float32).reshape(128, 1),
        "qblk": qblk.astype(BF16),
        "vc4": vc4.astype(BF16),
        "e4": e4.astype(BF16),
        "msel": msel,
        "r4": r4,
        "ecls": e_cls.astype(np.float32).reshape(4, 1),
        "eps": np.full((4, 1), 1e-5, np.float32),
        "ynb": (cls + ob).astype(np.float32).reshape(128, 1),
        "wo": ow.astype(BF16),
        "lng": np.tile(ln_g, (4, 1)),
        "lnb": np.tile(ln_b, (4, 1)),
    }

    ew32 = ew.astype(np.float32)
    smaj = plan['smaj']
    caps = plan['caps']
    slot_off = plan['slot_off']
    SL = plan['SL']
    wpairs = plan['wpairs']
    NPAD = NW * PW
    in_maps = []
    for c in range(NCORES):
        e = plan['eidx'][c]
        sm = smaj[e]
        ch = sm // CH
        starts = np.searchsorted(ch, np.arange(NCH))
        rank = np.arange(len(e)) - starts[ch]
        slots = slot_off[ch] + rank
        g_idx = np.zeros(SL, np.int64)
        sl_dst = np.full(SL, DSENT, np.float32)
        sl_ew = np.zeros(SL, np.float32)
        g_idx[slots] = src[e] - c * NPC
        sl_dst[slots] = (sm % CH).astype(np.float32)
        sl_ew[slots] = ew32[e]
        dstl_pad = np.full((128, NPAD), DSENT, np.float32)
        eww_pad = np.zeros((128, NPAD), np.float32)
        for w in range(NW):
            for jl, (bl, k) in enumerate(wpairs[w]):
                blo = (plan['wbase'][w] // 128 + bl) * 128
                klo, khi = slot_off[k], slot_off[k] + caps[k]
                lo = max(blo, klo) - blo
                hi = min(blo + 128, khi) - blo
                col = w * PW + jl
                dstl_pad[lo:hi, col] = sl_dst[blo + lo:blo + hi]
                eww_pad[lo:hi, col] = sl_ew[blo + lo:blo + hi]

        nids_own = node_ids[c * NPC:(c + 1) * NPC]
        ids_e_mask = (src // NPC) == c
        # layer-0 weighted count matrix over vocab for OWN dsts
        own_dst_mask = (dst // NPC) == c
        ids_e = node_ids[src[own_dst_mask]]
        dl_e = dst[own_dst_mask] - c * NPC
        Cf = np.bincount(dl_e * N + ids_e, weights=ew[own_dst_mask],
                         minlength=NPC * N).reshape(NPC, N).astype(np.float32)
        CtT = Cf.T.astype(FP8)
        ct_tiles = CtT.reshape(N // 128, 128, NPC).transpose(1, 0, 2).copy()
        m = dict(shared)
        dsth_pad = np.floor(dstl_pad / DL)
        dstl8_pad = dstl_pad - dsth_pad * DL
        m.update({
            "h0w1own": (T1.astype(np.float32)[nids_own]
                        + b1[None, :].astype(np.float32)).astype(BF16),
            "ct": ct_tiles,
            "idx12": _wrap16(g_idx),
            "dsth": dsth_pad.astype(BF16),
            "dstl8": dstl8_pad.astype(BF16),
            "eww": eww_pad.astype(BF16),
        })
        in_maps.append(m)
    return in_maps


def kernel(**inputs):
    from concourse.bass_utils import run_bass_kernel_spmd
    plan = _build_plan(inputs["src"], inputs["dst"])
    nc = _build_program(plan)
    in_maps = _host_prep(inputs, plan)
    res = run_bass_kernel_spmd(nc, in_maps, core_ids=list(range(NCORES)))
    y = np.concatenate([res.results[c]["y"] for c in range(NCORES)], axis=0)
    return np.ascontiguousarray(y.astype(np.float32))
